# revision 38
# baseline (speedup 1.0000x reference)
"""NetVLAD layer on 8 Trainium2 NeuronCores (Bass/Tile), v5.

Problem: descriptors [B=16, D=512, N=4096] f32, W [K=64, D], b [K],
centers [D, K].
  scores = softmax_K(W @ desc + b)            [B, K, N]
  agg[b,d,k] = sum_n scores[b,k,n] desc[b,d,n]
  vlad = agg - centers * sum_n(scores);  intra-L2-norm over D; global L2.

Sharding: data-parallel over B across 8 cores (2 items per core);
W/b/centers replicated.

v5 design (v1 121.6us -> v3 103.6 -> v4 58.8us):
  - desc pre-cast fp8e4m3 on host in BOTH layouts ([d,n] and [n,d]),
    8.4 MB/core, streamed as 16 half-strip jobs (512 n each, the two
    batch items interleaved) on the sync + scalar HWDGE queues; consts
    ride the otherwise-idle gpsimd SWDGE queue.
  - per job: scores [64k, 512n] = 2 fp8 DoubleRow matmuls (W
    stationary, 256-deep d per pass); one ACT exp with per-partition
    bias -> bf16; 4 PE transposes -> [128, 4, K] PSUM; softmax as 3 DVE
    ops (3D reduce -> Z, reciprocal, broadcast multiply -> softT fp8);
    agg[K, D] += 2 fp8 DoubleRow matmuls (2 n-chunks each); ssum[K, 1]
    += 2 DoubleRow ones-column matmuls.  Per-item PSUM accumulators
    (separate banks -- start_tensor_calc pending-zero is bank-wide).
  - software pipeline: transposes run 1 job behind mm1, mm2 2 jobs
    behind, so PE always has independent work while a job's softmax
    chain completes; fine job granularity keeps the post-DMA drain
    short.
  - tail per item, emitted right after that item's last mm2: vlad via
    scalar_tensor_tensor, row sumsq via ACT Square+accum_out, rn =
    exp(-0.5 ln ss) (all ACT funcs share a table with Exp), output
    scaled by rn * 0.125 (global L2 over unit columns is exactly
    sqrt(K)) and stored bf16 (host casts back to f32).
"""

import sys

sys.path.insert(0, "/opt/trn_rl_repo")

import numpy as np
import ml_dtypes

B, D, K, N = 16, 512, 64, 4096
N_CORES = 8
B_PER = B // N_CORES           # 2 items per core
DT = D // 128                  # 4 d-tiles
NJ = 8                         # half-strip jobs per item (512 n each)
NH = N // NJ                   # 512 columns per job
CPJ = NH // 128                # 4 n-chunks of 128 per job

_CACHE = {}


def _build():
    import concourse.bass as bass  # noqa: F401
    import concourse.tile as tile
    from concourse import bacc, mybir
    from contextlib import ExitStack

    bf16 = mybir.dt.bfloat16
    f8 = mybir.dt.float8e4
    f32 = mybir.dt.float32
    AF = mybir.ActivationFunctionType
    OP = mybir.AluOpType
    AX = mybir.AxisListType
    DR = mybir.MatmulPerfMode.DoubleRow

    nc = bacc.Bacc("TRN2", target_bir_lowering=False, debug=False,
                   num_devices=N_CORES)

    # per-strip blocks, one 4 KB row per partition
    da_d = nc.dram_tensor("da", [B_PER, NJ // 2, 128, DT, 2 * NH], f8,
                          kind="ExternalInput").ap()
    dt_d = nc.dram_tensor("dt", [B_PER, NJ // 2, 128, 2 * CPJ, 512], f8,
                          kind="ExternalInput").ap()
    wt_d = nc.dram_tensor("wt", [128, DT, K], f8, kind="ExternalInput").ap()
    eye_d = nc.dram_tensor("eye", [64, 64], bf16,
                           kind="ExternalInput").ap()
    cnegb_d = nc.dram_tensor("cnegb", [K, 1 + D], f32,
                             kind="ExternalInput").ap()
    out_d = nc.dram_tensor("out", [B_PER, K, D], bf16,
                           kind="ExternalOutput").ap()

    with tile.TileContext(nc) as tc, ExitStack() as ctx:
        const = ctx.enter_context(tc.tile_pool(name="const", bufs=1))
        sdesc = ctx.enter_context(tc.tile_pool(name="sdesc", bufs=3))
        sdt = ctx.enter_context(tc.tile_pool(name="sdt", bufs=4))
        pexp = ctx.enter_context(tc.tile_pool(name="pexp", bufs=3))
        psoft = ctx.enter_context(tc.tile_pool(name="psoft", bufs=3))
        small = ctx.enter_context(tc.tile_pool(name="small", bufs=16))
        med = ctx.enter_context(tc.tile_pool(name="med", bufs=2))
        # PSUM bank budget (8): sc 2 + xt 2 + agg 2 + ss 2
        ps_sc = ctx.enter_context(tc.tile_pool(name="ps_sc", bufs=2,
                                               space="PSUM"))
        ps_xt = ctx.enter_context(tc.tile_pool(name="ps_xt", bufs=2,
                                               space="PSUM"))
        ps_agg = ctx.enter_context(tc.tile_pool(name="ps_agg", bufs=2,
                                                space="PSUM"))
        ps_ss = ctx.enter_context(tc.tile_pool(name="ps_ss", bufs=2,
                                               space="PSUM"))

        # ---- constants: few big-row DMAs so the scalar ring's data
        # stream is not delayed by hundreds of tiny descriptors ----
        wt_sb = const.tile([128, DT, K], f8, tag="wt")
        nc.scalar.dma_start(out=wt_sb[:], in_=wt_d[:])
        eye_sb = const.tile([64, 64], bf16, tag="eye")
        nc.scalar.dma_start(out=eye_sb[:], in_=eye_d[:])
        cnegb_sb = const.tile([K, 1 + D], f32, tag="cnegb")
        nc.scalar.dma_start(out=cnegb_sb[:], in_=cnegb_d[:])
        b_sb = cnegb_sb[:, 0:1]
        cneg_sb = cnegb_sb[:, 1:1 + D]
        ones2_sb = const.tile([128, 2, 1], f8, tag="ones2")
        nc.vector.memset(ones2_sb[:], 1.0)

        agg_tiles = [ps_agg.tile([K, D], f32, tag="agg", name=f"agg{i}")
                     for i in range(B_PER)]
        ss_tiles = [ps_ss.tile([K, 1], f32, tag="ss", name=f"ss{i}")
                    for i in range(B_PER)]

        pend_exp = {i: [] for i in range(B_PER)}
        pend_tr = []   # (i, [(j, exp, dT), (j+1, exp, dT)]) pairs
        pend_mm2 = []  # (i, pair, soft_g) awaiting mm2

        def emit_tr(grp):
            i, pair = grp
            j0 = pair[0][0]
            w = CPJ * len(pair)
            xt = ps_xt.tile([128, w, K], bf16, tag="xt",
                            name=f"xt{i}_{j0}")
            for h, (j, exp_h, dTt, cb) in enumerate(pair):
                for cc in range(CPJ):
                    nc.tensor.transpose(
                        xt[:, CPJ * h + cc, :],
                        exp_h[:, 128 * cc:128 * (cc + 1)],
                        eye_sb[:],
                    )
            z8 = small.tile([128, w], f32, tag="z", name=f"z{i}_{j0}")
            nc.vector.reduce_sum(z8[:], xt[:], axis=AX.X)
            r8 = small.tile([128, w], f32, tag="r", name=f"r{i}_{j0}")
            nc.vector.reciprocal(r8[:], z8[:])
            soft_g = psoft.tile([128, w, K], f8, tag="soft",
                                name=f"soft{i}_{j0}")
            nc.vector.tensor_mul(
                soft_g[:], xt[:],
                r8[:, :, None].broadcast_to((128, w, K)))
            pend_mm2.append((i, pair, soft_g))

        def emit_mm2(grp):
            i, pair, soft_g = grp
            for h, (j, exp_h, dTt, cb) in enumerate(pair):
                for p in range(CPJ // 2):
                    nc.tensor.matmul(
                        agg_tiles[i][:],
                        lhsT=soft_g[:, CPJ * h + 2 * p:CPJ * h + 2 * p + 2, :],
                        rhs=dTt[:, cb + 2 * p:cb + 2 * p + 2, :],
                        perf_mode=DR,
                        start=(j == 0 and p == 0),
                        stop=(j == NJ - 1 and p == CPJ // 2 - 1))
            for h, (j, exp_h, dTt, cb) in enumerate(pair):
                for p in range(CPJ // 2):
                    nc.tensor.matmul(
                        ss_tiles[i][:],
                        lhsT=soft_g[:, CPJ * h + 2 * p:CPJ * h + 2 * p + 2, :],
                        rhs=ones2_sb[:], perf_mode=DR,
                        start=(j == 0 and p == 0),
                        stop=(j == NJ - 1 and p == CPJ // 2 - 1))

        def emit_tail(i):
            vlad_sb = med.tile([K, D], f32, tag="vlad", name=f"vlad{i}")
            nc.vector.scalar_tensor_tensor(
                vlad_sb[:], in0=cneg_sb, scalar=ss_tiles[i][:],
                in1=agg_tiles[i][:], op0=OP.mult, op1=OP.add,
            )
            # row sumsq: ACT Square+accum for item 0, DVE mul+reduce for
            # item 1 so the two tails run on disjoint engines
            sq_sb = med.tile([K, D], f32, tag="sq", name=f"sq{i}")
            ss_sb = small.tile([K, 1], f32, tag="ss2", name=f"ss2{i}")
            if i == 0:
                nc.scalar.activation(sq_sb[:], vlad_sb[:], func=AF.Square,
                                     accum_out=ss_sb[:])
            else:
                nc.vector.tensor_mul(sq_sb[:], vlad_sb[:], vlad_sb[:])
                nc.vector.reduce_sum(ss_sb[:], sq_sb[:], axis=AX.X)
            ln_sb = small.tile([K, 1], f32, tag="ln", name=f"ln{i}")
            nc.scalar.activation(ln_sb[:], ss_sb[:], func=AF.Ln)
            rn_sb = small.tile([K, 1], f32, tag="rn", name=f"rn{i}")
            nc.scalar.activation(rn_sb[:], ln_sb[:], func=AF.Exp,
                                 scale=-0.5)
            # intra-norm by rn; global norm is exactly 1/sqrt(K) = 0.125
            outT_sb = med.tile([K, D], bf16, tag="outT", name=f"outT{i}")
            nc.vector.tensor_scalar(out=outT_sb[:], in0=vlad_sb[:],
                                    scalar1=rn_sb[:], scalar2=0.125,
                                    op0=OP.mult, op1=OP.mult)
            nc.sync.dma_start(out=out_d[i], in_=outT_sb[:])

        for s in range(NJ // 2):
            for i in range(B_PER):
                dbf = sdesc.tile([128, DT, 2 * NH], f8, tag="dbf",
                                 name=f"dbf{i}_{s}")
                nc.sync.dma_start(out=dbf[:], in_=da_d[i, s])
                dTt = sdt.tile([128, 2 * CPJ, 512], f8, tag="dT",
                               name=f"dT{i}_{s}")
                nc.scalar.dma_start(out=dTt[:], in_=dt_d[i, s])
                pair = []
                for u in range(2):
                    j = 2 * s + u
                    # mm1: scores [64k, 512n], fp8 DoubleRow, W stationary
                    scp = ps_sc.tile([64, 512], f32, tag="sc",
                                     name=f"sc{i}_{j}")
                    for T in range(2):
                        nc.tensor.matmul(
                            scp[:],
                            lhsT=wt_sb[:, 2 * T:2 * T + 2, :],
                            rhs=dbf[:, 2 * T:2 * T + 2,
                                    NH * u:NH * (u + 1)],
                            perf_mode=DR, start=(T == 0), stop=(T == 1))
                    # exp(scores + b) -> bf16
                    exp_h = pexp.tile([64, 512], bf16, tag="exps",
                                      name=f"exps{i}_{j}")
                    nc.scalar.activation(out=exp_h[:], in_=scp[:],
                                         func=AF.Exp, bias=b_sb,
                                         scale=1.0)
                    pair.append((j, exp_h, dTt, CPJ * u))
                if s == NJ // 2 - 1:
                    pend_tr.append((i, [pair[0]]))
                    pend_tr.append((i, [pair[1]]))
                else:
                    pend_tr.append((i, pair))
                # software pipeline: transposes 1 strip behind, mm2 2 behind
                if len(pend_tr) > 1:
                    emit_tr(pend_tr.pop(0))
                if len(pend_mm2) > 1:
                    emit_mm2(pend_mm2.pop(0))
        while pend_tr:
            emit_tr(pend_tr.pop(0))
        while pend_mm2:
            grp = pend_mm2.pop(0)
            emit_mm2(grp)
            emit_tail(grp[0])

    nc.compile()
    return nc


def _get_nc():
    if "nc" not in _CACHE:
        _CACHE["nc"] = _build()
    return _CACHE["nc"]


def _host_inputs(descriptors, W, b, centers):
    f8 = ml_dtypes.float8_e4m3fn
    d16 = np.asarray(descriptors, dtype=np.float32).astype(f8)  # [B, D, N]
    wt = np.ascontiguousarray(
        W.astype(np.float32).T.reshape(DT, 128, K).transpose(1, 0, 2)
    ).astype(f8)                                       # [128, DT, K] p-major
    eye = np.eye(64, dtype=np.float32).astype(ml_dtypes.bfloat16)
    cnegb = np.ascontiguousarray(np.concatenate(
        [b.astype(np.float32).reshape(K, 1),
         -centers.astype(np.float32).T], axis=1))      # [K, 1+D]
    common = {"wt": wt, "eye": eye, "cnegb": cnegb}
    in_maps = []
    for core in range(N_CORES):
        dc = d16[B_PER * core:B_PER * (core + 1)]        # [2, D, N] fp8
        # da[i, s, p, t, x] = desc[i, 128t+p, 1024s+x]
        da = dc.reshape(B_PER, DT, 128, NJ // 2, 2 * NH
                        ).transpose(0, 3, 2, 1, 4)
        # dt[i, s, p, c, d] = desc[i, d, 1024s+128c+p]
        dt_ = dc.reshape(B_PER, D, NJ // 2, 2 * CPJ, 128
                         ).transpose(0, 2, 4, 3, 1)
        m = dict(common)
        m["da"] = np.ascontiguousarray(da)
        m["dt"] = np.ascontiguousarray(dt_)
        in_maps.append(m)
    return in_maps


def _run(inputs, trace=False):
    from concourse.bass_utils import run_bass_kernel_spmd

    descriptors = np.asarray(inputs["descriptors"])
    W = np.asarray(inputs["W"])
    b = np.asarray(inputs["b"])
    centers = np.asarray(inputs["centers"])
    nc = _get_nc()
    in_maps = _host_inputs(descriptors, W, b, centers)
    res = run_bass_kernel_spmd(nc, in_maps, list(range(N_CORES)), trace=trace)
    outs = []
    for core in range(N_CORES):
        o = np.asarray(res.results[core]["out"], dtype=np.float32)
        outs.append(np.transpose(o, (0, 2, 1)).reshape(B_PER, D * K))
    full = np.concatenate(outs, axis=0).astype(np.float32)
    return full, res


def kernel(**inputs):
    out, _ = _run(inputs, trace=False)
    return out


if __name__ == "__main__":
    rng = np.random.default_rng(0)
    inputs = {
        "descriptors": rng.standard_normal((B, D, N), dtype=np.float32),
        "W": (rng.standard_normal((K, D)) * 0.05).astype(np.float32),
        "b": (rng.standard_normal((K,)) * 0.05).astype(np.float32),
        "centers": rng.standard_normal((D, K)).astype(np.float32),
    }
    out = kernel(**inputs)
    print("out shape:", out.shape, out.dtype)
